# revision 7
# baseline (speedup 1.0000x reference)
"""Causal self-attention on 8 Trainium2 NeuronCores.

Sharding: core c handles batch b = c//2 and head-group g = c%2 (8 of 16
heads). Per core: qkv projection for its head slice (fp32r matmuls),
causal attention (exp softmax without max-subtraction — scores are
N(0,1)-scaled, no overflow risk), pairwise AllGather of the per-group
attention output y between the two cores of a batch, then c_proj with
output columns sharded by group. Host only slices inputs / concatenates
outputs.
"""

import numpy as np

B, T, C, H = 4, 2048, 1024, 16
D = C // H            # 64
NCORES = 8
GROUPS = [[0, 1], [2, 3], [4, 5], [6, 7]]
QT = 512              # q-tile width (matmul moving dim)
KB = 128              # k-block size (PSUM partition dim)
NQT = T // QT         # 4
HPAIRS = 4            # head pairs per core (8 heads)

_CACHE = {}


# --------------------------------------------------------------------------
# walrus workaround: this toolchain allows only ONE sync-wait per
# instruction. Split the end-of-kernel drain, and hoist excess waits from
# any instruction onto NoOps inserted just before it (same engine).
# --------------------------------------------------------------------------
def _patched_tc_class():
    import concourse.tile as tile
    from concourse.vector_clock import ScopedClock, VectorClock

    class PatchedTileContext(tile.TileContext):
        def _drain_and_barrier(self, tick_clock, wait_clock):
            gc = tick_clock.global_clock
            n = len(gc)
            ahead = [p for p in range(n) if gc[p] > 0]
            for p in ahead:
                vec = [gc[q] if q == p else 0 for q in range(n)]
                inst = self.nc.sync.drain()
                wait_clock.add_sem_waits(
                    inst.ins, ScopedClock({None: VectorClock(vec)})
                )
            if not ahead:
                inst = self.nc.sync.drain()
                wait_clock.add_sem_waits(
                    inst.ins, ScopedClock({None: tick_clock.global_clock})
                )
            self.nc.all_engine_barrier()
            assert self.sems is not None
            popped = self.nc._tile_sem_poison_stack.pop()
            assert popped is self._sem_poison
            self.nc.clear_and_free_semaphores(list(self.sems.allocated().values()))
            self.nc.all_engine_barrier()

    return PatchedTileContext


def _split_sync_waits(nc, max_waits=1):
    import concourse.mybir as mybir

    k = 0
    for f in nc.m.functions:
        for bb in f.blocks:
            newl = []
            dirty = False
            for inst in bb.instructions:
                si = inst.sync_info
                if si is not None and len(si.on_wait) > max_waits:
                    waits = list(si.on_wait)
                    excess, keep = waits[:-max_waits], waits[-max_waits:]
                    for w in excess:
                        k += 1
                        nop = mybir.InstNoOp(
                            name=f"I-waitsplit-{k}", ins=[], outs=[]
                        )
                        nop.engine = inst.engine
                        nop.sync_info = mybir.SyncInfo(on_wait=[w], on_update=[])
                        newl.append(nop)
                    inst.sync_info = mybir.SyncInfo(
                        on_wait=keep, on_update=si.on_update
                    )
                    dirty = True
                newl.append(inst)
            if dirty:
                bb.instructions = newl
    return k


# --------------------------------------------------------------------------
# the Bass program (identical on all 8 cores; only input data differs)
# --------------------------------------------------------------------------
def _build_nc(split_waits=True):
    import concourse.bass as bass
    import concourse.mybir as mybir

    F32 = mybir.dt.float32
    F32R = mybir.dt.float32r
    EXP = mybir.ActivationFunctionType.Exp
    IDENT = mybir.ActivationFunctionType.Identity
    COPY = mybir.ActivationFunctionType.Copy
    MULT = mybir.AluOpType.mult
    ADD = mybir.AluOpType.add

    PatchedTileContext = _patched_tc_class()

    nc = bass.Bass()

    # ---- parameters --------------------------------------------------
    xT_p = nc.declare_dram_parameter("xT", [C, T], F32R, isOutput=False)
    wqk_p = nc.declare_dram_parameter("wqk", [C, 1024], F32R, isOutput=False)
    wv_p = nc.declare_dram_parameter("wv", [C, 512], F32R, isOutput=False)
    wp_p = nc.declare_dram_parameter("wp", [C, 512], F32R, isOutput=False)
    bqk_p = nc.declare_dram_parameter("bqk", [128, 8], F32, isOutput=False)
    bv_p = nc.declare_dram_parameter("bv", [1, 512], F32R, isOutput=False)
    bp_p = nc.declare_dram_parameter("bp", [1, 512], F32R, isOutput=False)
    mask_p = nc.declare_dram_parameter("masks", [4, 128, QT], F32R, isOutput=False)
    out_p = nc.declare_dram_parameter("out", [T, 512], F32, isOutput=True)


    with PatchedTileContext(nc) as tc:
        dram_cm = tc.tile_pool(name="dramp", bufs=1, space="DRAM")
        dram = dram_cm.__enter__()
        # internal DRAM for the pairwise allgather of y^T [features, T]
        y_own = dram.tile([512, T], F32R, name="y_own", tag="y_own")
        y_all = dram.tile([1024, T], F32R, name="y_all", tag="y_all")
        persist_cm = tc.tile_pool(name="persist", bufs=1)
        persist = persist_cm.__enter__()
        qv_cm = tc.tile_pool(name="qv", bufs=1)
        qv = qv_cm.__enter__()

        # ---- persistent small tensors -------------------------------
        masks = [
            persist.tile([128, QT], F32R, name=f"mask{m}", tag=f"mask{m}")
            for m in range(4)
        ]
        for m in range(4):
            nc.sync.dma_start(masks[m][:], mask_p[m])
        bqk_sb = persist.tile([128, 8], F32)
        nc.sync.dma_start(bqk_sb[:], bqk_p[:])
        bv_sb = persist.tile([1, 512], F32R)
        nc.sync.dma_start(bv_sb[:], bv_p[:])
        bp_sb = persist.tile([1, 512], F32R)
        nc.sync.dma_start(bp_sb[:], bp_p[:])
        ones_row = persist.tile([1, 128], F32R)
        nc.vector.memset(ones_row[:].bitcast(F32), 1.0)
        bv_b = persist.tile([128, 512], F32R)   # bv broadcast to 128 partitions
        bp_b = persist.tile([128, 512], F32R)   # bp broadcast

        # ---- persistent activations ---------------------------------
        # qk_sb[ft]: feature-tile ft of [Q^T | K^T], [128, T]; ft 0..3 = Q
        # (head pair ft), ft 4..7 = K.
        qk_sb = [qv.tile([128, T], F32R, name=f"qk{ft}", tag=f"qk{ft}") for ft in range(8)]
        # V_sb[tt]: [128, 8, 65] — T-chunk tt of V per local head + ones col
        v_sb = [qv.tile([128, 8, 65], F32R, name=f"v{tt}", tag=f"v{tt}") for tt in range(16)]
        for tt in range(16):
            nc.vector.memset(v_sb[tt][:, :, 64].bitcast(F32), 1.0)

        # ================= phase B/C: projections ====================
        with (
            tc.tile_pool(name="proj", bufs=1) as proj,
            tc.tile_pool(name="ps_qk", bufs=3, space="PSUM") as ps_qk,
            tc.tile_pool(name="ps_v", bufs=4, space="PSUM") as ps_v,
            tc.tile_pool(name="ps_bc", bufs=1, space="PSUM") as ps_bc,
        ):
            # bias broadcasts via K=1 matmul (ones_row.T @ bias_row)
            bcv = ps_bc.tile([128, 512], F32, tag="bc")
            nc.tensor.matmul(bcv[:], ones_row[:], bv_sb[:], start=True, stop=True)
            nc.scalar.activation(bv_b[:], bcv[:], COPY)
            bcp = ps_bc.tile([128, 512], F32, tag="bc")
            nc.tensor.matmul(bcp[:], ones_row[:], bp_sb[:], start=True, stop=True)
            nc.scalar.activation(bp_b[:], bcp[:], COPY)

            wqk_sb = [proj.tile([128, 1024], F32R, name=f"wqk{kc}", tag=f"wqk{kc}") for kc in range(8)]
            wv_sb = [proj.tile([128, 512], F32R, name=f"wv{kc}", tag=f"wv{kc}") for kc in range(8)]
            for kc in range(8):
                nc.sync.dma_start(wqk_sb[kc][:], wqk_p[kc * 128 : (kc + 1) * 128, :])
                nc.sync.dma_start(wv_sb[kc][:], wv_p[kc * 128 : (kc + 1) * 128, :])

            for th in range(2):  # T halves of 1024
                t0 = th * 1024
                xt_sb = [
                    proj.tile([128, 1024], F32R, name=f"xt{th}_{kc}", tag=f"xt{kc}")
                    for kc in range(8)
                ]
                for kc in range(8):
                    nc.sync.dma_start(
                        xt_sb[kc][:],
                        xT_p[kc * 128 : (kc + 1) * 128, t0 : t0 + 1024],
                    )
                # B: Q^T/K^T tiles (transposed-out): out [feat 128, T 512]
                for ft in range(8):
                    for tt in range(2):
                        ps = ps_qk.tile([128, QT], F32, tag="qkps")
                        for kc in range(8):
                            nc.tensor.matmul(
                                ps[:],
                                wqk_sb[kc][:, ft * 128 : (ft + 1) * 128],
                                xt_sb[kc][:, tt * QT : (tt + 1) * QT],
                                start=(kc == 0),
                                stop=(kc == 7),
                            )
                        # bias add (per-partition scalar) + copy to SBUF
                        nc.scalar.activation(
                            qk_sb[ft][:, t0 + tt * QT : t0 + (tt + 1) * QT],
                            ps[:],
                            IDENT,
                            bias=bqk_sb[:, ft : ft + 1],
                        )
                # C: V tiles (normal-out): out [T 128, feat 512]
                for i in range(8):
                    tt16 = th * 8 + i
                    ps = ps_v.tile([128, 512], F32, tag="vps")
                    for kc in range(8):
                        nc.tensor.matmul(
                            ps[:],
                            xt_sb[kc][:, i * 128 : (i + 1) * 128],
                            wv_sb[kc][:],
                            start=(kc == 0),
                            stop=(kc == 7),
                        )
                    nc.vector.tensor_tensor(
                        out=v_sb[tt16][:, :, 0:64],
                        in0=ps[:].rearrange("p (h d) -> p h d", h=8),
                        in1=bv_b[:].rearrange("p (h d) -> p h d", h=8),
                        op=ADD,
                    )

        # ================= phase D: attention ========================
        y_pool_cm = tc.tile_pool(name="ypool", bufs=1)
        y_pool = y_pool_cm.__enter__()
        # y_sb[hp]: [128, T] f32r — normalized attention out, heads 2hp/2hp+1
        y_sb = [y_pool.tile([128, T], F32R, name=f"y{hp}", tag=f"y{hp}") for hp in range(4)]

        with (
            tc.tile_pool(name="attn", bufs=1) as attn,
            tc.tile_pool(name="ps_d", bufs=2, space="PSUM") as ps_d,
        ):
            for qt in range(NQT):
                q0 = qt * QT
                for hp in range(HPAIRS):
                    nkb = 4 * qt + 4
                    ya = ps_d.tile([65, QT], F32, tag="YA")
                    yb = ps_d.tile([65, QT], F32, tag="YB")
                    for kb in range(nkb):
                        sa = ps_d.tile([128, QT], F32, tag="SA")
                        sb = ps_d.tile([128, QT], F32, tag="SB")
                        # S^T = K^T.T-slice @ Q^T (contraction d=64),
                        # 2 heads packed into row groups (0,0)/(64,0)
                        nc.tensor.matmul(
                            sa[:],
                            qk_sb[4 + hp][0:64, kb * KB : (kb + 1) * KB],
                            qk_sb[hp][0:64, q0 : q0 + QT],
                            start=True,
                            stop=True,
                        )
                        nc.tensor.matmul(
                            sb[:],
                            qk_sb[4 + hp][64:128, kb * KB : (kb + 1) * KB],
                            qk_sb[hp][64:128, q0 : q0 + QT],
                            start=True,
                            stop=True,
                        )
                        pa = attn.tile([128, QT], F32R, tag="PA", bufs=3)
                        pb = attn.tile([128, QT], F32R, tag="PB", bufs=3)
                        nc.scalar.activation(pa[:], sa[:], EXP)
                        nc.scalar.activation(pb[:], sb[:], EXP)
                        m = kb - 4 * qt
                        if m >= 0:  # diagonal block: causal mask (multiplicative)
                            nc.vector.tensor_tensor(
                                out=pa[:], in0=pa[:], in1=masks[m], op=MULT
                            )
                            nc.vector.tensor_tensor(
                                out=pb[:], in0=pb[:], in1=masks[m], op=MULT
                            )
                        # Y^T += V'.T @ P^T  (V' has ones column -> row 64 = denom)
                        nc.tensor.matmul(
                            ya[:],
                            v_sb[kb][:, 2 * hp, :],
                            pa[:],
                            start=(kb == 0),
                            stop=(kb == nkb - 1),
                        )
                        nc.tensor.matmul(
                            yb[:],
                            v_sb[kb][:, 2 * hp + 1, :],
                            pb[:],
                            start=(kb == 0),
                            stop=(kb == nkb - 1),
                        )
                    # normalize: y = Y[0:64] * (1/Y[64]) broadcast via PE
                    for head_half, yps in ((0, ya), (1, yb)):
                        rr = attn.tile([1, QT], F32R, tag="rr", bufs=2)
                        with nc.allow_low_precision(reason="softmax recip"):
                            nc.vector.reciprocal(rr[:], yps[64:65, :])
                        bc = ps_d.tile([64, QT], F32, tag="SA" if head_half == 0 else "SB")
                        nc.tensor.matmul(
                            bc[:], ones_row[:, 0:64], rr[:], start=True, stop=True
                        )
                        cc = attn.tile([64, QT], F32R, tag="cc", bufs=2)
                        nc.scalar.activation(cc[:], bc[:], COPY)
                        nc.vector.tensor_tensor(
                            out=y_sb[hp][
                                head_half * 64 : (head_half + 1) * 64, q0 : q0 + QT
                            ],
                            in0=yps[0:64, :],
                            in1=cc[:],
                            op=MULT,
                        )

        # ================= phase E: pairwise allgather of y ==========
        for hp in range(4):
            nc.sync.dma_start(y_own[hp * 128 : (hp + 1) * 128, :], y_sb[hp][:])
        nc.gpsimd.collective_compute(
            "AllGather",
            mybir.AluOpType.bypass,
            replica_groups=GROUPS,
            ins=[y_own[:].opt()],
            outs=[y_all[:].opt()],
        )
        y_pool_cm.__exit__(None, None, None)

        # ================= phase F: c_proj ===========================
        with (
            tc.tile_pool(name="cproj", bufs=1) as cp,
            tc.tile_pool(name="ps_f", bufs=4, space="PSUM") as ps_f,
        ):
            yt_sb = [cp.tile([128, T], F32R, name=f"yt{kc}", tag=f"yt{kc}") for kc in range(8)]
            wp_sb = [cp.tile([128, 512], F32R, name=f"wp{kc}", tag=f"wp{kc}") for kc in range(8)]
            for kc in range(8):
                nc.sync.dma_start(yt_sb[kc][:], y_all[kc * 128 : (kc + 1) * 128, :])
                nc.sync.dma_start(wp_sb[kc][:], wp_p[kc * 128 : (kc + 1) * 128, :])
            for tn in range(16):
                ps = ps_f.tile([128, 512], F32, tag="fps")
                for kc in range(8):
                    nc.tensor.matmul(
                        ps[:],
                        yt_sb[kc][:, tn * 128 : (tn + 1) * 128],
                        wp_sb[kc][:],
                        start=(kc == 0),
                        stop=(kc == 7),
                    )
                ot = cp.tile([128, 512], F32, tag="ot", bufs=3)
                nc.vector.tensor_tensor(
                    out=ot[:], in0=ps[:], in1=bp_b[:].bitcast(F32), op=ADD
                )
                nc.sync.dma_start(out_p[tn * 128 : (tn + 1) * 128, :], ot[:])

        qv_cm.__exit__(None, None, None)
        persist_cm.__exit__(None, None, None)
        dram_cm.__exit__(None, None, None)

    if split_waits:
        _split_sync_waits(nc)
    return nc


# --------------------------------------------------------------------------
# host side
# --------------------------------------------------------------------------
def _make_masks():
    i = np.arange(128)[:, None]
    j = np.arange(QT)[None, :]
    return np.stack(
        [(i + 128 * m <= j).astype(np.float32) for m in range(4)]
    )  # [4, 128, QT]


def _prep_core_inputs(x, w_attn, b_attn, w_proj, b_proj):
    masks = _make_masks()
    in_maps = []
    for c in range(NCORES):
        b, g = divmod(c, 2)
        sl = slice(512 * g, 512 * (g + 1))
        wq = w_attn[:, 0 * C :][:, sl] * 0.125  # fold 1/sqrt(D)
        wk = w_attn[:, C : 2 * C][:, sl]
        bq = b_attn[0 * C :][sl] * 0.125
        bk = b_attn[C : 2 * C][sl]
        wqk = np.concatenate([wq, wk], axis=1)          # [C, 1024]
        bqk = np.concatenate([bq, bk]).reshape(8, 128).T  # [128, 8]
        in_maps.append(
            {
                "xT": np.ascontiguousarray(x[b].T).astype(np.float32),
                "wqk": np.ascontiguousarray(wqk).astype(np.float32),
                "wv": np.ascontiguousarray(w_attn[:, 2 * C :][:, sl]).astype(
                    np.float32
                ),
                "wp": np.ascontiguousarray(w_proj[:, sl]).astype(np.float32),
                "bqk": np.ascontiguousarray(bqk).astype(np.float32),
                "bv": b_attn[2 * C :][sl].reshape(1, 512).astype(np.float32),
                "bp": b_proj[sl].reshape(1, 512).astype(np.float32),
                "masks": masks,
            }
        )
    return in_maps


def _get_runner():
    if "runner" in _CACHE:
        return _CACHE["runner"]
    nc = _build_nc()
    _CACHE["nc"] = nc
    _CACHE["runner"] = nc
    return nc


def run_spmd(in_maps):
    from concourse.bass_utils import run_bass_kernel_spmd

    nc = _get_runner()
    return run_bass_kernel_spmd(nc, in_maps, core_ids=list(range(NCORES)))


def kernel(x, w_attn, b_attn, w_proj, b_proj):
    x = np.asarray(x, dtype=np.float32)
    w_attn = np.asarray(w_attn, dtype=np.float32)
    b_attn = np.asarray(b_attn, dtype=np.float32)
    w_proj = np.asarray(w_proj, dtype=np.float32)
    b_proj = np.asarray(b_proj, dtype=np.float32)

    in_maps = _prep_core_inputs(x, w_attn, b_attn, w_proj, b_proj)
    res = run_spmd(in_maps)
    out = np.empty((B, T, C), dtype=np.float32)
    for b in range(B):
        out[b, :, 0:512] = res.results[2 * b]["out"]
        out[b, :, 512:1024] = res.results[2 * b + 1]["out"]
    return out


# revision 8
# speedup vs baseline: 29.9245x; 29.9245x over previous
"""Causal self-attention on 8 Trainium2 NeuronCores.

Sharding: core c handles batch b = c//2 and head-group g = c%2 (8 of 16
heads). Per core: qkv projection for its head slice (fp32r matmuls),
causal attention (exp softmax without max-subtraction — scores are
N(0,1)-scaled, no overflow risk), pairwise AllGather of the per-group
attention output y between the two cores of a batch, then c_proj with
output columns sharded by group. Host only slices inputs / concatenates
outputs.
"""

import numpy as np

B, T, C, H = 4, 2048, 1024, 16
D = C // H            # 64
NCORES = 8
GROUPS = [[0, 1], [2, 3], [4, 5], [6, 7]]
QT = 512              # q-tile width (matmul moving dim)
KB = 128              # k-block size (PSUM partition dim)
NQT = T // QT         # 4
HPAIRS = 4            # head pairs per core (8 heads)

_CACHE = {}


# --------------------------------------------------------------------------
# walrus workaround: this toolchain allows only ONE sync-wait per
# instruction. Split the end-of-kernel drain, and hoist excess waits from
# any instruction onto NoOps inserted just before it (same engine).
# --------------------------------------------------------------------------
def _patched_tc_class():
    import concourse.tile as tile
    from concourse.vector_clock import ScopedClock, VectorClock

    class PatchedTileContext(tile.TileContext):
        def _drain_and_barrier(self, tick_clock, wait_clock):
            gc = tick_clock.global_clock
            n = len(gc)
            ahead = [p for p in range(n) if gc[p] > 0]
            for p in ahead:
                vec = [gc[q] if q == p else 0 for q in range(n)]
                inst = self.nc.sync.drain()
                wait_clock.add_sem_waits(
                    inst.ins, ScopedClock({None: VectorClock(vec)})
                )
            if not ahead:
                inst = self.nc.sync.drain()
                wait_clock.add_sem_waits(
                    inst.ins, ScopedClock({None: tick_clock.global_clock})
                )
            self.nc.all_engine_barrier()
            assert self.sems is not None
            popped = self.nc._tile_sem_poison_stack.pop()
            assert popped is self._sem_poison
            self.nc.clear_and_free_semaphores(list(self.sems.allocated().values()))
            self.nc.all_engine_barrier()

    return PatchedTileContext


def _split_sync_waits(nc, max_waits=1):
    import concourse.mybir as mybir

    k = 0
    for f in nc.m.functions:
        for bb in f.blocks:
            newl = []
            dirty = False
            for inst in bb.instructions:
                si = inst.sync_info
                if si is not None and len(si.on_wait) > max_waits:
                    waits = list(si.on_wait)
                    excess, keep = waits[:-max_waits], waits[-max_waits:]
                    for w in excess:
                        k += 1
                        nop = mybir.InstNoOp(
                            name=f"I-waitsplit-{k}", ins=[], outs=[]
                        )
                        nop.engine = inst.engine
                        nop.sync_info = mybir.SyncInfo(on_wait=[w], on_update=[])
                        newl.append(nop)
                    inst.sync_info = mybir.SyncInfo(
                        on_wait=keep, on_update=si.on_update
                    )
                    dirty = True
                newl.append(inst)
            if dirty:
                bb.instructions = newl
    return k


# --------------------------------------------------------------------------
# the Bass program (identical on all 8 cores; only input data differs)
# --------------------------------------------------------------------------
def _build_nc(split_waits=True):
    import concourse.bass as bass
    import concourse.mybir as mybir

    F32 = mybir.dt.float32
    F32R = mybir.dt.float32r
    EXP = mybir.ActivationFunctionType.Exp
    IDENT = mybir.ActivationFunctionType.Identity
    COPY = mybir.ActivationFunctionType.Copy
    MULT = mybir.AluOpType.mult
    ADD = mybir.AluOpType.add

    PatchedTileContext = _patched_tc_class()

    nc = bass.Bass()

    # ---- parameters --------------------------------------------------
    xT_p = nc.declare_dram_parameter("xT", [C, T], F32R, isOutput=False)
    wqk_p = nc.declare_dram_parameter("wqk", [C, 1024], F32R, isOutput=False)
    wv_p = nc.declare_dram_parameter("wv", [C, 512], F32R, isOutput=False)
    wp_p = nc.declare_dram_parameter("wp", [C, 512], F32R, isOutput=False)
    bqk_p = nc.declare_dram_parameter("bqk", [128, 8], F32, isOutput=False)
    bv_p = nc.declare_dram_parameter("bv", [1, 512], F32R, isOutput=False)
    bp_p = nc.declare_dram_parameter("bp", [1, 512], F32R, isOutput=False)
    mask_p = nc.declare_dram_parameter("masks", [4, 128, QT], F32R, isOutput=False)
    out_p = nc.declare_dram_parameter("out", [T, 512], F32, isOutput=True)


    with PatchedTileContext(nc) as tc:
        dram_cm = tc.tile_pool(name="dramp", bufs=1, space="DRAM")
        dram = dram_cm.__enter__()
        # internal DRAM for the pairwise allgather of y^T [features, T]
        y_own = dram.tile([512, T], F32R, name="y_own", tag="y_own")
        y_all = dram.tile([1024, T], F32R, name="y_all", tag="y_all")
        persist_cm = tc.tile_pool(name="persist", bufs=1)
        persist = persist_cm.__enter__()
        qv_cm = tc.tile_pool(name="qv", bufs=1)
        qv = qv_cm.__enter__()

        # ---- persistent small tensors -------------------------------
        masks = [
            persist.tile([128, QT], F32R, name=f"mask{m}", tag=f"mask{m}")
            for m in range(4)
        ]
        for m in range(4):
            nc.sync.dma_start(masks[m][:], mask_p[m])
        bqk_sb = persist.tile([128, 8], F32)
        nc.sync.dma_start(bqk_sb[:], bqk_p[:])
        bv_sb = persist.tile([1, 512], F32R)
        nc.sync.dma_start(bv_sb[:], bv_p[:])
        bp_sb = persist.tile([1, 512], F32R)
        nc.sync.dma_start(bp_sb[:], bp_p[:])
        ones_row = persist.tile([1, 128], F32R)
        nc.vector.memset(ones_row[:].bitcast(F32), 1.0)
        bv_b = persist.tile([128, 512], F32R)   # bv broadcast to 128 partitions
        bp_b = persist.tile([128, 512], F32R)   # bp broadcast

        # ---- persistent activations ---------------------------------
        # qk_sb[ft]: feature-tile ft of [Q^T | K^T], [128, T]; ft 0..3 = Q
        # (head pair ft), ft 4..7 = K.
        qk_sb = [qv.tile([128, T], F32R, name=f"qk{ft}", tag=f"qk{ft}") for ft in range(8)]
        # V_sb[tt]: [128, 8, 65] — T-chunk tt of V per local head + ones col
        v_sb = [qv.tile([128, 8, 65], F32R, name=f"v{tt}", tag=f"v{tt}") for tt in range(16)]
        for tt in range(16):
            nc.vector.memset(v_sb[tt][:, :, 64].bitcast(F32), 1.0)

        # ================= phase B/C: projections ====================
        with (
            tc.tile_pool(name="proj", bufs=1) as proj,
            tc.tile_pool(name="ps_qk", bufs=3, space="PSUM") as ps_qk,
            tc.tile_pool(name="ps_v", bufs=4, space="PSUM") as ps_v,
            tc.tile_pool(name="ps_bc", bufs=1, space="PSUM") as ps_bc,
        ):
            # bias broadcasts via K=1 matmul (ones_row.T @ bias_row)
            bcv = ps_bc.tile([128, 512], F32, tag="bc")
            nc.tensor.matmul(bcv[:], ones_row[:], bv_sb[:], start=True, stop=True)
            nc.scalar.activation(bv_b[:], bcv[:], COPY)
            bcp = ps_bc.tile([128, 512], F32, tag="bc")
            nc.tensor.matmul(bcp[:], ones_row[:], bp_sb[:], start=True, stop=True)
            nc.scalar.activation(bp_b[:], bcp[:], COPY)

            wqk_sb = [proj.tile([128, 1024], F32R, name=f"wqk{kc}", tag=f"wqk{kc}") for kc in range(8)]
            wv_sb = [proj.tile([128, 512], F32R, name=f"wv{kc}", tag=f"wv{kc}") for kc in range(8)]
            for kc in range(8):
                nc.sync.dma_start(wqk_sb[kc][:], wqk_p[kc * 128 : (kc + 1) * 128, :])
                nc.sync.dma_start(wv_sb[kc][:], wv_p[kc * 128 : (kc + 1) * 128, :])

            for th in range(2):  # T halves of 1024
                t0 = th * 1024
                xt_sb = [
                    proj.tile([128, 1024], F32R, name=f"xt{th}_{kc}", tag=f"xt{kc}")
                    for kc in range(8)
                ]
                for kc in range(8):
                    nc.sync.dma_start(
                        xt_sb[kc][:],
                        xT_p[kc * 128 : (kc + 1) * 128, t0 : t0 + 1024],
                    )
                # B: Q^T/K^T tiles (transposed-out): out [feat 128, T 512]
                for ft in range(8):
                    for tt in range(2):
                        ps = ps_qk.tile([128, QT], F32, tag="qkps")
                        for kc in range(8):
                            nc.tensor.matmul(
                                ps[:],
                                wqk_sb[kc][:, ft * 128 : (ft + 1) * 128],
                                xt_sb[kc][:, tt * QT : (tt + 1) * QT],
                                start=(kc == 0),
                                stop=(kc == 7),
                            )
                        # bias add (per-partition scalar) + copy to SBUF
                        nc.scalar.activation(
                            qk_sb[ft][:, t0 + tt * QT : t0 + (tt + 1) * QT],
                            ps[:],
                            IDENT,
                            bias=bqk_sb[:, ft : ft + 1],
                        )
                # C: V tiles (normal-out): out [T 128, feat 512]
                for i in range(8):
                    tt16 = th * 8 + i
                    ps = ps_v.tile([128, 512], F32, tag="vps")
                    for kc in range(8):
                        nc.tensor.matmul(
                            ps[:],
                            xt_sb[kc][:, i * 128 : (i + 1) * 128],
                            wv_sb[kc][:],
                            start=(kc == 0),
                            stop=(kc == 7),
                        )
                    nc.vector.tensor_tensor(
                        out=v_sb[tt16][:, :, 0:64],
                        in0=ps[:].rearrange("p (h d) -> p h d", h=8),
                        in1=bv_b[:].rearrange("p (h d) -> p h d", h=8),
                        op=ADD,
                    )

        # ================= phase D: attention ========================
        y_pool_cm = tc.tile_pool(name="ypool", bufs=1)
        y_pool = y_pool_cm.__enter__()
        # y_sb[hp]: [128, T] f32r — normalized attention out, heads 2hp/2hp+1
        y_sb = [y_pool.tile([128, T], F32R, name=f"y{hp}", tag=f"y{hp}") for hp in range(4)]

        with (
            tc.tile_pool(name="attn", bufs=1) as attn,
            tc.tile_pool(name="ps_d", bufs=2, space="PSUM") as ps_d,
        ):
            for qt in range(NQT):
                q0 = qt * QT
                for hp in range(HPAIRS):
                    nkb = 4 * qt + 4
                    ya = ps_d.tile([65, QT], F32, tag="YA")
                    yb = ps_d.tile([65, QT], F32, tag="YB")
                    for kb in range(nkb):
                        sa = ps_d.tile([128, QT], F32, tag="SA")
                        sb = ps_d.tile([128, QT], F32, tag="SB")
                        # S^T = K^T.T-slice @ Q^T (contraction d=64),
                        # 2 heads packed into row groups (0,0)/(64,0)
                        nc.tensor.matmul(
                            sa[:],
                            qk_sb[4 + hp][0:64, kb * KB : (kb + 1) * KB],
                            qk_sb[hp][0:64, q0 : q0 + QT],
                            start=True,
                            stop=True,
                        )
                        nc.tensor.matmul(
                            sb[:],
                            qk_sb[4 + hp][64:128, kb * KB : (kb + 1) * KB],
                            qk_sb[hp][64:128, q0 : q0 + QT],
                            start=True,
                            stop=True,
                        )
                        pa = attn.tile([128, QT], F32R, tag="PA", bufs=3)
                        pb = attn.tile([128, QT], F32R, tag="PB", bufs=3)
                        nc.scalar.activation(pa[:], sa[:], EXP)
                        nc.scalar.activation(pb[:], sb[:], EXP)
                        m = kb - 4 * qt
                        if m >= 0:  # diagonal block: causal mask (multiplicative)
                            nc.vector.tensor_tensor(
                                out=pa[:], in0=pa[:], in1=masks[m], op=MULT
                            )
                            nc.vector.tensor_tensor(
                                out=pb[:], in0=pb[:], in1=masks[m], op=MULT
                            )
                        # Y^T += V'.T @ P^T  (V' has ones column -> row 64 = denom)
                        nc.tensor.matmul(
                            ya[:],
                            v_sb[kb][:, 2 * hp, :],
                            pa[:],
                            start=(kb == 0),
                            stop=(kb == nkb - 1),
                        )
                        nc.tensor.matmul(
                            yb[:],
                            v_sb[kb][:, 2 * hp + 1, :],
                            pb[:],
                            start=(kb == 0),
                            stop=(kb == nkb - 1),
                        )
                    # normalize: y = Y[0:64] * (1/Y[64]) broadcast via PE
                    for head_half, yps in ((0, ya), (1, yb)):
                        rr = attn.tile([1, QT], F32R, tag="rr", bufs=2)
                        with nc.allow_low_precision(reason="softmax recip"):
                            nc.vector.reciprocal(rr[:], yps[64:65, :])
                        bc = ps_d.tile([64, QT], F32, tag="SA" if head_half == 0 else "SB")
                        nc.tensor.matmul(
                            bc[:], ones_row[:, 0:64], rr[:], start=True, stop=True
                        )
                        cc = attn.tile([64, QT], F32R, tag="cc", bufs=2)
                        nc.scalar.activation(cc[:], bc[:], COPY)
                        nc.vector.tensor_tensor(
                            out=y_sb[hp][
                                head_half * 64 : (head_half + 1) * 64, q0 : q0 + QT
                            ],
                            in0=yps[0:64, :],
                            in1=cc[:],
                            op=MULT,
                        )

        # ================= phase E: pairwise allgather of y ==========
        for hp in range(4):
            nc.sync.dma_start(y_own[hp * 128 : (hp + 1) * 128, :], y_sb[hp][:])
        nc.gpsimd.collective_compute(
            "AllGather",
            mybir.AluOpType.bypass,
            replica_groups=GROUPS,
            ins=[y_own[:].opt()],
            outs=[y_all[:].opt()],
        )
        y_pool_cm.__exit__(None, None, None)

        # ================= phase F: c_proj ===========================
        with (
            tc.tile_pool(name="cproj", bufs=1) as cp,
            tc.tile_pool(name="ps_f", bufs=4, space="PSUM") as ps_f,
        ):
            yt_sb = [cp.tile([128, T], F32R, name=f"yt{kc}", tag=f"yt{kc}") for kc in range(8)]
            wp_sb = [cp.tile([128, 512], F32R, name=f"wp{kc}", tag=f"wp{kc}") for kc in range(8)]
            for kc in range(8):
                nc.sync.dma_start(yt_sb[kc][:], y_all[kc * 128 : (kc + 1) * 128, :])
                nc.sync.dma_start(wp_sb[kc][:], wp_p[kc * 128 : (kc + 1) * 128, :])
            for tn in range(16):
                ps = ps_f.tile([128, 512], F32, tag="fps")
                for kc in range(8):
                    nc.tensor.matmul(
                        ps[:],
                        yt_sb[kc][:, tn * 128 : (tn + 1) * 128],
                        wp_sb[kc][:],
                        start=(kc == 0),
                        stop=(kc == 7),
                    )
                ot = cp.tile([128, 512], F32, tag="ot", bufs=3)
                nc.vector.tensor_tensor(
                    out=ot[:], in0=ps[:], in1=bp_b[:].bitcast(F32), op=ADD
                )
                nc.sync.dma_start(out_p[tn * 128 : (tn + 1) * 128, :], ot[:])

        qv_cm.__exit__(None, None, None)
        persist_cm.__exit__(None, None, None)
        dram_cm.__exit__(None, None, None)

    if split_waits:
        _split_sync_waits(nc)
    return nc


# --------------------------------------------------------------------------
# host side
# --------------------------------------------------------------------------
def _make_masks():
    i = np.arange(128)[:, None]
    j = np.arange(QT)[None, :]
    return np.stack(
        [(i + 128 * m <= j).astype(np.float32) for m in range(4)]
    )  # [4, 128, QT]


def _prep_core_inputs(x, w_attn, b_attn, w_proj, b_proj):
    masks = _make_masks()
    in_maps = []
    for c in range(NCORES):
        b, g = divmod(c, 2)
        sl = slice(512 * g, 512 * (g + 1))
        wq = w_attn[:, 0 * C :][:, sl] * 0.125  # fold 1/sqrt(D)
        wk = w_attn[:, C : 2 * C][:, sl]
        bq = b_attn[0 * C :][sl] * 0.125
        bk = b_attn[C : 2 * C][sl]
        wqk = np.concatenate([wq, wk], axis=1)          # [C, 1024]
        bqk = np.concatenate([bq, bk]).reshape(8, 128).T  # [128, 8]
        in_maps.append(
            {
                "xT": np.ascontiguousarray(x[b].T).astype(np.float32),
                "wqk": np.ascontiguousarray(wqk).astype(np.float32),
                "wv": np.ascontiguousarray(w_attn[:, 2 * C :][:, sl]).astype(
                    np.float32
                ),
                "wp": np.ascontiguousarray(w_proj[:, sl]).astype(np.float32),
                "bqk": np.ascontiguousarray(bqk).astype(np.float32),
                "bv": b_attn[2 * C :][sl].reshape(1, 512).astype(np.float32),
                "bp": b_proj[sl].reshape(1, 512).astype(np.float32),
                "masks": masks,
            }
        )
    return in_maps


def _make_compiled(nc):
    """Build a reusable jitted SPMD callable (mirrors
    bass2jax.run_bass_via_pjrt's multi-core branch, but cached so repeat
    calls don't re-trace)."""
    import jax
    import concourse.mybir as mybir
    from jax.experimental.shard_map import shard_map
    from jax.sharding import Mesh, PartitionSpec
    from concourse import bass2jax

    bass2jax.install_neuronx_cc_hook()
    partition_name = (
        nc.partition_id_tensor.name if nc.partition_id_tensor else None
    )
    in_names, out_names, out_avals, zero_shapes = [], [], [], []
    for alloc in nc.m.functions[0].allocations:
        if not isinstance(alloc, mybir.MemoryLocationSet):
            continue
        name = alloc.memorylocations[0].name
        if alloc.kind == "ExternalInput":
            if name != partition_name:
                in_names.append(name)
        elif alloc.kind == "ExternalOutput":
            out_names.append(name)
            shape = tuple(alloc.tensor_shape)
            dtype = mybir.dt.np(alloc.dtype)
            out_avals.append(jax.core.ShapedArray(shape, dtype))
            zero_shapes.append((shape, dtype))
    n_params = len(in_names)
    in_names_full = list(in_names) + list(out_names)
    if partition_name is not None:
        in_names_full.append(partition_name)
    donate = tuple(range(n_params, n_params + len(out_names)))

    def _body(*args):
        operands = list(args)
        if partition_name is not None:
            operands.append(bass2jax.partition_id_tensor())
        outs = bass2jax._bass_exec_p.bind(
            *operands,
            out_avals=tuple(out_avals),
            in_names=tuple(in_names_full),
            out_names=tuple(out_names),
            lowering_input_output_aliases=(),
            sim_require_finite=True,
            sim_require_nnan=True,
            nc=nc,
        )
        return tuple(outs)

    devices = jax.devices()[:NCORES]
    mesh = Mesh(np.asarray(devices), ("core",))
    in_specs = (PartitionSpec("core"),) * (n_params + len(out_names))
    out_specs = (PartitionSpec("core"),) * len(out_names)
    sharded = jax.jit(
        shard_map(
            _body, mesh=mesh, in_specs=in_specs, out_specs=out_specs,
            check_rep=False,
        ),
        donate_argnums=donate,
        keep_unused=True,
    )
    return {
        "sharded": sharded,
        "in_names": in_names,
        "out_names": out_names,
        "out_avals": out_avals,
        "zero_shapes": zero_shapes,
        "mesh": mesh,
    }


def _get_compiled():
    if "compiled" not in _CACHE:
        _CACHE["compiled"] = _make_compiled(_build_nc())
    return _CACHE["compiled"]


def _concat_inputs(cc, in_maps):
    arrs = []
    for name in cc["in_names"]:
        arrs.append(
            np.concatenate([np.asarray(m[name]) for m in in_maps], axis=0)
        )
    return arrs


def _zeros(cc):
    return [
        np.zeros((NCORES * shape[0], *shape[1:]), dtype)
        for shape, dtype in cc["zero_shapes"]
    ]


def run_spmd(in_maps):
    """Returns an object with .results: list of per-core {name: array}."""
    cc = _get_compiled()
    out_arrs = cc["sharded"](*_concat_inputs(cc, in_maps), *_zeros(cc))
    results = []
    for c in range(NCORES):
        d = {}
        for i, name in enumerate(cc["out_names"]):
            shape = cc["out_avals"][i].shape
            d[name] = np.asarray(out_arrs[i]).reshape(NCORES, *shape)[c]
        results.append(d)

    class _R:
        pass

    r = _R()
    r.results = results
    return r


def kernel(x, w_attn, b_attn, w_proj, b_proj):
    x = np.asarray(x, dtype=np.float32)
    w_attn = np.asarray(w_attn, dtype=np.float32)
    b_attn = np.asarray(b_attn, dtype=np.float32)
    w_proj = np.asarray(w_proj, dtype=np.float32)
    b_proj = np.asarray(b_proj, dtype=np.float32)

    in_maps = _prep_core_inputs(x, w_attn, b_attn, w_proj, b_proj)
    res = run_spmd(in_maps)
    out = np.empty((B, T, C), dtype=np.float32)
    for b in range(B):
        out[b, :, 0:512] = res.results[2 * b]["out"]
        out[b, :, 512:1024] = res.results[2 * b + 1]["out"]
    return out
